# revision 3
# baseline (speedup 1.0000x reference)
"""ConvLSTM2D forward on 8 Trainium2 NeuronCores — v3 (z-layout epilogue).

v3 vs v2: the gate-planar deinterleave (12 DMAs) + s2 interleave (4 DMAs) are
replaced by 3 partition-shift DMAs: gates i, f and sigmoid(zc) are shifted to
the o-gate's partitions 64:96 and the whole LSTM cell update runs in z-layout
[32, 8(tau), 256] there (FD=2048 DVE ops).  DMA issue is spread across the
Sync, Scalar(ACT) and GpSimd descriptor engines (~0.75us serialized issue cost
per dma_start per engine was the v2 bottleneck: 22 DMAs/step on one queue).
Gate order in M is (i, f, o, c).
"""

import numpy as np
import ml_dtypes

import concourse.bacc as bacc
import concourse.bass as bass
import concourse.mybir as mybir
import concourse.tile as tile
from concourse import bass_utils

F32 = mybir.dt.float32
BF16 = mybir.dt.bfloat16
AF = mybir.ActivationFunctionType
OP = mybir.AluOpType
BF = ml_dtypes.bfloat16

B, T, H, W, CIN = 8, 10, 256, 256, 8
G = 4
RT = 32
HIN = RT + 2
NTAU = H // RT
GMAP = [0, 1, 3, 2]            # M-gate-column (i,f,o,c) -> ref gate (i,f,c,o)
GSCALE = [0.2, 0.2, 0.2, 1.0]  # hard-sigmoid prescale for i,f,o


def pack_inputs(x, Wx, Wh, b):
    x = np.asarray(x, dtype=np.float32)
    Wx = np.asarray(Wx, dtype=np.float32)
    Wh = np.asarray(Wh, dtype=np.float32)
    b = np.asarray(b, dtype=np.float32)

    W9 = np.concatenate([Wx, Wh], axis=2)  # [3,3,9,4]
    wb = np.zeros((103, 9, 128), dtype=np.float32)
    r = np.arange(RT)
    for cg in range(3):
        for cc in range(3):
            ch = cg * 3 + cc
            for kh in range(3):
                for kw in range(3):
                    for g in range(G):
                        wb[cc * HIN + r + kh, cg * 3 + kw, g * RT + r] = \
                            W9[kh, kw, ch, GMAP[g]] * GSCALE[g]
    for g in range(G):
        bias = 0.2 * b[GMAP[g]] + 0.5 if g < 3 else b[GMAP[g]]
        wb[102, 0, g * RT + r] = bias
    wbh = wb.astype(BF)

    xks = []
    for bi in range(B):
        xpad = np.zeros((T, H + 2, W + 2, CIN), dtype=np.float32)
        xpad[:, 1:H + 1, 1:W + 1, :] = x[bi]
        sw = np.lib.stride_tricks.sliding_window_view(xpad, HIN, axis=1)
        sw = sw[:, ::RT]                     # [T, tau, w, c, lr]
        arr = sw.transpose(0, 3, 4, 1, 2)    # [T, c, lr, tau, w]
        arr = arr.reshape(T, CIN * HIN, NTAU, W + 2)
        xk = np.empty((T, 273, NTAU, W + 2), dtype=BF)
        xk[:, 0:102] = arr[:, 0:102]
        xk[:, 102] = np.float32(1.0)
        xk[:, 103:205] = arr[:, 102:204]
        xk[:, 205:273] = arr[:, 204:272]
        xks.append(xk)
    return xks, wbh


def build_program():
    nc = bacc.Bacc("TRN2", target_bir_lowering=False, debug=False)
    xk_d = nc.dram_tensor("xk", [T, 273, NTAU, W + 2], BF16, kind="ExternalInput")
    wb_d = nc.dram_tensor("wb", [103, 9, 128], BF16, kind="ExternalInput")
    out_d = nc.dram_tensor("out", [H, W], BF16, kind="ExternalOutput")

    with tile.TileContext(nc) as tc:
        with tc.tile_pool(name="wpool", bufs=1) as wpool, \
             tc.tile_pool(name="xpool", bufs=3) as xpool, \
             tc.tile_pool(name="state", bufs=1) as state, \
             tc.tile_pool(name="zpool", bufs=2) as zpool, \
             tc.tile_pool(name="zpsum", bufs=2, space="PSUM") as zpsum:

            wt = wpool.tile([103, 9, 128], BF16, tag="wt")
            nc.sync.dma_start(out=wt, in_=wb_d.ap())

            # persistent: cell state in z-layout at partitions 64:96
            cz = state.tile([96, NTAU, W], BF16, tag="cz")
            nc.vector.memset(cz, 0.0)
            x2t = [state.tile([102, NTAU, W + 2], BF16, tag=f"x2_{i}",
                              name=f"x2_{i}")
                   for i in range(2)]
            nc.vector.memset(x2t[0], 0.0)
            nc.vector.memset(x2t[1], 0.0)
            hzt = [state.tile([96, NTAU, W + 2], BF16, tag=f"hz_{i}",
                              name=f"hz_{i}")
                   for i in range(2)]
            nc.vector.memset(hzt[0], 0.0)
            nc.vector.memset(hzt[1], 0.0)

            def mm4(zp_t, pi, rhs, kw, start, stop):
                lhsT = wt[:, pi, :] if pi < 3 else wt[0:102, pi, :]
                for tp in range(4):
                    nc.tensor.matmul(
                        zp_t[:, 2 * tp:2 * tp + 2, :], lhsT,
                        rhs[:, 2 * tp:2 * tp + 2, kw:kw + W],
                        start=start, stop=stop)

            xat = {}
            zpt = {}

            def load_x(s):
                xa0 = xpool.tile([103, NTAU, W + 2], BF16, tag="xa0")
                xa1 = xpool.tile([102, NTAU, W + 2], BF16, tag="xa1")
                nc.gpsimd.dma_start(out=xa0, in_=xk_d[s, 0:103])
                nc.gpsimd.dma_start(out=xa1, in_=xk_d[s, 103:205])
                xat[s] = (xa0, xa1)

            def load_x2(s):
                nc.gpsimd.dma_start(out=x2t[s % 2][0:68], in_=xk_d[s, 205:273])

            def x_mms_a(s):
                zp_t = zpsum.tile([128, NTAU, W], F32, tag="zp")
                zpt[s] = zp_t
                xa0, _ = xat[s]
                for kw in range(3):
                    mm4(zp_t, kw, xa0, kw, start=(kw == 0), stop=False)

            def x_mms_b(s):
                _, xa1 = xat[s]
                for kw in range(3):
                    mm4(zpt[s], 3 + kw, xa1, kw, start=False, stop=False)

            def h_mms(s):
                for kw in range(3):
                    mm4(zpt[s], 6 + kw, x2t[s % 2], kw,
                        start=False, stop=(kw == 2))

            load_x(0)
            load_x(1)
            load_x2(0)
            load_x2(1)
            x_mms_a(0)
            x_mms_b(0)
            h_mms(0)
            x_mms_a(1)

            for t in range(T):
                zp_t = zpt.pop(t)
                # evac: clip(i,f,o) on DVE || sigmoid(c) on ACT
                zhs = zpool.tile([96, NTAU, W], BF16, tag="zhs")
                nc.vector.tensor_scalar(out=zhs, in0=zp_t[0:96],
                                        scalar1=0.0, scalar2=1.0,
                                        op0=OP.max, op1=OP.min)
                i_s = zpool.tile([96, NTAU, W], BF16, tag="i_s")
                f_s = zpool.tile([96, NTAU, W], BF16, tag="f_s")
                nc.sync.dma_start(out=i_s[64:96], in_=zhs[0:32])
                nc.sync.dma_start(out=f_s[64:96], in_=zhs[32:64])
                zsc = zpool.tile([128, NTAU, W], BF16, tag="zsc")
                nc.scalar.activation(out=zsc[96:128], in_=zp_t[96:128],
                                     func=AF.Sigmoid)
                # keep tensor busy through the epilogue: finish step t+1's
                # x-accum, then start step t+2's
                if t + 1 < T:
                    x_mms_b(t + 1)
                if t + 2 < T:
                    load_x(t + 2)
                    load_x2(t + 2)
                    x_mms_a(t + 2)
                # shift sc to partitions 64:96 (one affine DMA)
                sc_s = zpool.tile([96, NTAU, W], BF16, tag="sc_s")
                nc.scalar.dma_start(out=sc_s[64:96], in_=zsc[96:128])
                # cell update in z-layout at partitions 64:96, pipelined in
                # tau-halves so half A's sigmoid/h/scatter overlap half B's
                # DVE chain and the scatter ring drain.
                t1 = zpool.tile([96, NTAU, W], BF16, tag="t1")
                t2 = zpool.tile([96, NTAU, W], BF16, tag="t2")
                s2z = zpool.tile([96, NTAU, W], BF16, tag="s2z")
                hz = hzt[t % 2]
                x2n = x2t[(t + 1) % 2] if t < T - 1 else None
                for a, b in ((0, 4), (4, 8)):
                    nc.vector.tensor_tensor(out=t2[64:96, a:b],
                                            in0=f_s[64:96, a:b],
                                            in1=cz[64:96, a:b], op=OP.mult)
                    nc.vector.tensor_tensor(out=t1[64:96, a:b],
                                            in0=i_s[64:96, a:b],
                                            in1=sc_s[64:96, a:b], op=OP.mult)
                    nc.vector.tensor_tensor(out=cz[64:96, a:b],
                                            in0=t1[64:96, a:b],
                                            in1=t2[64:96, a:b], op=OP.add)
                    nc.scalar.activation(out=s2z[64:96, a:b],
                                         in_=cz[64:96, a:b], func=AF.Sigmoid)
                    nc.vector.tensor_tensor(out=hz[64:96, a:b, 1:W + 1],
                                            in0=zhs[64:96, a:b],
                                            in1=s2z[64:96, a:b], op=OP.mult)
                    if x2n is not None:
                        eng = nc.sync if a == 0 else nc.scalar
                        eng.dma_start(out=x2n[69:101, a:b], in_=hz[64:96, a:b])
                        eng.dma_start(
                            out=x2n[101:102, max(a - 1, 0):b - 1],
                            in_=hz[64:65, max(a, 1):b])
                        eng.dma_start(
                            out=x2n[68:69, max(a, 1):b],
                            in_=hz[95:96, max(a - 1, 0):b - 1])
                if t == T - 1:
                    nc.sync.dma_start(
                        out=out_d.rearrange("(t j) w -> j t w", j=32),
                        in_=hz[64:96, :, 1:W + 1])
                if t + 1 < T:
                    h_mms(t + 1)
    nc.compile()
    return nc


_CACHE = {}


def _get_program():
    if "nc" not in _CACHE:
        _CACHE["nc"] = build_program()
    return _CACHE["nc"]


def kernel(x, Wx, Wh, b, _run_opts=None):
    x = np.asarray(x, dtype=np.float32)
    Bn = x.shape[0]
    xks, wbh = pack_inputs(x, Wx, Wh, b)
    nc = _get_program()
    in_maps = [{"xk": np.ascontiguousarray(xks[bi]), "wb": wbh}
               for bi in range(Bn)]
    res = bass_utils.run_bass_kernel_spmd(
        nc, in_maps, core_ids=list(range(Bn)), **(_run_opts or {}))
    out = np.stack([np.asarray(res.results[bi]["out"], dtype=np.float32)
                    for bi in range(Bn)], axis=0)
    kernel.last_results = res
    return out[..., None]
